# revision 14
# baseline (speedup 1.0000x reference)
"""DND-LSTM (retrieval_knn) Trainium2 kernel — 8-core SPMD Bass/Tile implementation.

Strategy:
  - DND memory (400000 x 128 keys + vals) is row-sharded across 8 NeuronCores,
    padded to 50176 rows/core (pad keys = 1e4 so softmax weight underflows to 0).
  - Each core streams its key shard once: fused copy to keys_out + squared-L2
    distance per row (DVE subtract, ACT square, DVE grouped reduce).
  - Local softmax stats (min sqrt-dist, exp-sum, weighted val partial via
    TensorE matmul accumulation over the vals stream, also fused with copy-out).
  - One AllGather of the 130-float stats vector, replicated combine, then the
    tiny LSTM cell + actor/critic heads computed on-device.
  - Host pads/shards inputs and assembles the shifted keys_new/vals_new from
    the per-core output shards (the +1 row shift is pure indexing).
"""

import hashlib
import os
import shutil

import numpy as np

import concourse.bass as bass
import concourse.bacc as bacc
import concourse.mybir as mybir
import concourse.tile as tile
from concourse import masks
from concourse.bass_utils import run_bass_kernel_spmd

F32 = mybir.dt.float32
AF = mybir.ActivationFunctionType
ALU = mybir.AluOpType
AX = mybir.AxisListType

N_CORES = 8
D = 128
H = 128
NG = 4
OUT_DIM = 16
L = 400000
Q = 50176              # padded rows per core (392 * 128)
NCHUNK = Q // 128      # 392 rows-of-128 per core
PAD_VAL = 1.0e4
EPS = 1e-6

def _supertiles(cmax):
    """Split NCHUNK chunks-of-128 into supertiles of at most cmax chunks."""
    sts = []
    off = 0
    while off < NCHUNK:
        c = min(cmax, NCHUNK - off)
        sts.append((off * 128, c))
        off += c
    return sts


def _build_nc(cmax=32, stream_bufs=3, diff_bufs=2, sub_engine="vector",
              probe_no_collective=False, probe_no_tail=False):
    supertiles = _supertiles(cmax)
    nc = bacc.Bacc("TRN2", target_bir_lowering=False, debug=False,
                   num_devices=N_CORES)

    keys_in = nc.declare_dram_parameter("keys", [Q, D], F32, isOutput=False)
    vals_in = nc.declare_dram_parameter("vals", [Q, H], F32, isOutput=False)
    xr_in = nc.declare_dram_parameter("xr", [1, D], F32, isOutput=False)
    hr_in = nc.declare_dram_parameter("hr", [1, H], F32, isOutput=False)
    cr_in = nc.declare_dram_parameter("cr", [1, H], F32, isOutput=False)
    wi_in = nc.declare_dram_parameter("wi", [(NG + 1) * H, D], F32, isOutput=False)
    bi_in = nc.declare_dram_parameter("bi", [1, (NG + 1) * H], F32, isOutput=False)
    wh_in = nc.declare_dram_parameter("wh", [(NG + 1) * H, H], F32, isOutput=False)
    bh_in = nc.declare_dram_parameter("bh", [1, (NG + 1) * H], F32, isOutput=False)
    wa_in = nc.declare_dram_parameter("wa", [OUT_DIM, H], F32, isOutput=False)
    ba_in = nc.declare_dram_parameter("ba", [1, OUT_DIM], F32, isOutput=False)
    wc_in = nc.declare_dram_parameter("wc", [1, H], F32, isOutput=False)
    bc_in = nc.declare_dram_parameter("bc", [1, 1], F32, isOutput=False)

    keys_out = nc.declare_dram_parameter("keys_out", [Q, D], F32, isOutput=True)
    vals_out = nc.declare_dram_parameter("vals_out", [Q, H], F32, isOutput=True)
    outv = nc.declare_dram_parameter("outv", [1, OUT_DIM + 1 + 2 * H], F32,
                                     isOutput=True)

    cc_in = nc.dram_tensor("cc_in", [1, 2 + H], F32)
    cc_out = nc.dram_tensor("cc_out", [N_CORES, 2 + H], F32, addr_space="Shared")

    G4 = NG * H            # 512
    G5 = (NG + 1) * H      # 640

    with tile.TileContext(nc) as tc:
        with (
            tc.tile_pool(name="setup", bufs=1) as setup,
            tc.tile_pool(name="stream", bufs=stream_bufs) as stream,
            tc.tile_pool(name="diffp", bufs=diff_bufs) as diffp,
            tc.tile_pool(name="small", bufs=1) as small,
            tc.tile_pool(name="ps", bufs=2, space="PSUM") as ps,
            tc.tile_pool(name="psP", bufs=1, space="PSUM") as psP,
        ):
            # ---------------- setup: constants, weight transposes ----------
            ident = setup.tile([128, 128], F32)
            masks.make_identity(nc, ident[:])
            ones_col = setup.tile([128, 1], F32)
            nc.gpsimd.memset(ones_col[:], 1.0)
            ones_row = setup.tile([1, 128], F32)
            nc.gpsimd.memset(ones_row[:], 1.0)
            eps_bc = setup.tile([128, 1], F32)
            nc.gpsimd.memset(eps_bc[:], EPS)

            x_row = setup.tile([1, D], F32)
            nc.sync.dma_start(x_row[:], xr_in[:])
            h_row = setup.tile([1, H], F32)
            nc.sync.dma_start(h_row[:], hr_in[:])
            c_row = setup.tile([1, H], F32)
            nc.sync.dma_start(c_row[:], cr_in[:])

            # x as a column [128,1]
            p_xc = ps.tile([128, 1], F32, tag="ps")
            nc.tensor.transpose(p_xc[:], x_row[:], ident[0:1, 0:1])
            x_col = setup.tile([128, 1], F32)
            nc.vector.tensor_copy(x_col[:], p_xc[:])
            # h as a column
            p_hc = ps.tile([128, 1], F32, tag="ps")
            nc.tensor.transpose(p_hc[:], h_row[:], ident[0:1, 0:1])
            h_col = setup.tile([128, 1], F32)
            nc.vector.tensor_copy(h_col[:], p_hc[:])

            # x broadcast to all partitions, then tiled CMAX times along free
            p_xb = ps.tile([128, 128], F32, tag="ps")
            nc.tensor.matmul(p_xb[:], ones_row[:], x_row[:], start=True, stop=True)
            x_big = setup.tile([128, cmax, D], F32)
            nc.vector.tensor_copy(x_big[:, 0, :], p_xb[:])
            rep = 1
            while rep < cmax:
                n = min(rep, cmax - rep)
                nc.vector.tensor_copy(x_big[:, rep:rep + n, :], x_big[:, 0:n, :])
                rep += n

            # W_i2h.T and W_h2h.T in SBUF: [128(dim), 640]
            w_tmp = setup.tile([128, 5, D], F32, tag="w_tmp")
            nc.sync.dma_start(
                w_tmp[:], wi_in[:].rearrange("(g p) d -> p g d", p=128))
            wiT = setup.tile([128, G5], F32)
            for g in range(5):
                p_w = ps.tile([128, 128], F32, tag="ps")
                nc.tensor.transpose(p_w[:], w_tmp[:, g, :], ident[:])
                nc.vector.tensor_copy(wiT[:, g * 128:(g + 1) * 128], p_w[:])
            w_tmp2 = setup.tile([128, 5, H], F32, tag="w_tmp")
            nc.sync.dma_start(
                w_tmp2[:], wh_in[:].rearrange("(g p) d -> p g d", p=128))
            whT = setup.tile([128, G5], F32)
            for g in range(5):
                p_w = ps.tile([128, 128], F32, tag="ps")
                nc.tensor.transpose(p_w[:], w_tmp2[:, g, :], ident[:])
                nc.vector.tensor_copy(whT[:, g * 128:(g + 1) * 128], p_w[:])

            wa_sb = setup.tile([OUT_DIM, H], F32)
            nc.sync.dma_start(wa_sb[:], wa_in[:])
            p_wa = ps.tile([128, OUT_DIM], F32, tag="ps")
            nc.tensor.transpose(p_wa[:], wa_sb[:], ident[0:OUT_DIM, 0:OUT_DIM])
            waT = setup.tile([128, OUT_DIM], F32)
            nc.vector.tensor_copy(waT[:], p_wa[:])

            wc_sb = setup.tile([1, H], F32)
            nc.sync.dma_start(wc_sb[:], wc_in[:])
            p_wc = ps.tile([128, 1], F32, tag="ps")
            nc.tensor.transpose(p_wc[:], wc_sb[:], ident[0:1, 0:1])
            wcT = setup.tile([128, 1], F32)
            nc.vector.tensor_copy(wcT[:], p_wc[:])

            bi_sb = setup.tile([1, G5], F32)
            nc.sync.dma_start(bi_sb[:], bi_in[:])
            bh_sb = setup.tile([1, G5], F32)
            nc.sync.dma_start(bh_sb[:], bh_in[:])
            b_sum = setup.tile([1, G5], F32)
            nc.vector.tensor_add(b_sum[:], bi_sb[:], bh_sb[:])
            ba_sb = setup.tile([1, OUT_DIM], F32)
            nc.sync.dma_start(ba_sb[:], ba_in[:])
            bc_sb = setup.tile([1, 1], F32)
            nc.sync.dma_start(bc_sb[:], bc_in[:])

            # ---------------- LSTM gate preactivations (sigmoid table) -----
            p_g1 = ps.tile([1, G4], F32, tag="gates")
            nc.tensor.matmul(p_g1[:], x_col[:], wiT[:, 0:G4], start=True, stop=False)
            nc.tensor.matmul(p_g1[:], h_col[:], whT[:, 0:G4], start=False, stop=True)
            p_g2 = ps.tile([1, H], F32, tag="gates")
            nc.tensor.matmul(p_g2[:], x_col[:], wiT[:, G4:G5], start=True, stop=False)
            nc.tensor.matmul(p_g2[:], h_col[:], whT[:, G4:G5], start=False, stop=True)

            preact = small.tile([1, G5], F32)
            nc.vector.tensor_add(preact[:, 0:G4], p_g1[:], b_sum[:, 0:G4])
            nc.vector.tensor_add(preact[:, G4:G5], p_g2[:], b_sum[:, G4:G5])

            gates = small.tile([1, G4], F32)
            nc.scalar.activation(gates[:], preact[:, 0:G4], AF.Sigmoid)
            c_new = small.tile([1, H], F32)
            nc.scalar.activation(c_new[:], preact[:, G4:G5], AF.Tanh)

            f_g = gates[:, 0:H]
            i_g = gates[:, H:2 * H]
            o_g = gates[:, 2 * H:3 * H]
            r_g = gates[:, 3 * H:4 * H]

            t_fc = small.tile([1, H], F32)
            nc.vector.tensor_mul(t_fc[:], f_g, c_row[:])
            t_ic = small.tile([1, H], F32)
            nc.vector.tensor_mul(t_ic[:], i_g, c_new[:])
            c_pre = small.tile([1, H], F32)
            nc.vector.tensor_add(c_pre[:], t_fc[:], t_ic[:])

            # ---------------- phase A: stream keys ------------------------
            sims = small.tile([128, NCHUNK], F32)
            for (r0, C) in supertiles:
                kt = stream.tile([128, C, D], F32, tag="st")
                src = keys_in[r0:r0 + C * 128, :].rearrange(
                    "(p n) d -> p n d", p=128)
                nc.sync.dma_start(kt[:], src)
                dst = keys_out[r0:r0 + C * 128, :].rearrange(
                    "(p n) d -> p n d", p=128)
                nc.scalar.dma_start(dst, kt[:])

                diff = diffp.tile([128, C, D], F32, tag="diff")
                getattr(nc, sub_engine).tensor_sub(diff[:], kt[:], x_big[:, 0:C, :])
                nc.scalar.activation(diff[:], diff[:], AF.Square)
                off = r0 // 128
                nc.vector.tensor_reduce(
                    sims[:, off:off + C], diff[:], axis=AX.X, op=ALU.add)

            # ---------------- local softmax stats -------------------------
            sqs = small.tile([128, NCHUNK], F32)
            nc.scalar.activation(sqs[:], sims[:], AF.Sqrt, bias=eps_bc[:])
            rowmin = small.tile([128, 1], F32)
            nc.vector.tensor_reduce(rowmin[:], sqs[:], axis=AX.X, op=ALU.min)
            p_tr = ps.tile([1, 128], F32, tag="ps")
            nc.tensor.transpose(p_tr[:], rowmin[:], ident[:])
            m_sq = small.tile([1, 1], F32)
            nc.vector.tensor_reduce(m_sq[:], p_tr[:], axis=AX.X, op=ALU.min)
            # broadcast local min to [128,1]
            p_mb = ps.tile([128, 1], F32, tag="ps")
            nc.tensor.matmul(p_mb[:], ones_row[:], m_sq[:], start=True, stop=True)
            m_bc = small.tile([128, 1], F32)
            nc.vector.tensor_copy(m_bc[:], p_mb[:])

            e_w = small.tile([128, NCHUNK], F32)
            nc.scalar.activation(e_w[:], sqs[:], AF.Exp, bias=m_bc[:], scale=-1.0)
            srow = small.tile([128, 1], F32)
            nc.vector.tensor_reduce(srow[:], e_w[:], axis=AX.X, op=ALU.add)
            p_s = ps.tile([1, 1], F32, tag="ps")
            nc.tensor.matmul(p_s[:], ones_col[:], srow[:], start=True, stop=True)

            # ---------------- phase B: stream vals ------------------------
            p_P = psP.tile([1, H], F32, tag="P")
            ci = 0
            for (r0, C) in supertiles:
                vt = stream.tile([128, C, H], F32, tag="st")
                src = vals_in[r0:r0 + C * 128, :].rearrange(
                    "(p n) d -> p n d", p=128)
                nc.sync.dma_start(vt[:], src)
                dst = vals_out[r0:r0 + C * 128, :].rearrange(
                    "(p n) d -> p n d", p=128)
                nc.scalar.dma_start(dst, vt[:])
                off = r0 // 128
                for g in range(C):
                    nc.tensor.matmul(
                        p_P[:], e_w[:, off + g:off + g + 1], vt[:, g, :],
                        start=(ci == 0), stop=(ci == NCHUNK - 1))
                    ci += 1

            # ---------------- allgather stats ----------------------------
            stats = small.tile([1, 2 + H], F32)
            nc.vector.tensor_copy(stats[:, 0:1], m_sq[:])
            nc.vector.tensor_copy(stats[:, 1:2], p_s[:])
            nc.vector.tensor_copy(stats[:, 2:2 + H], p_P[:])
            nc.sync.dma_start(cc_in[:], stats[:])
            if not probe_no_collective:
                nc.gpsimd.collective_compute(
                    "AllGather",
                    ALU.bypass,
                    ins=[cc_in[:]],
                    outs=[cc_out[:]],
                    replica_groups=[list(range(N_CORES))],
                )
                stats8 = small.tile([N_CORES, 2 + H], F32)
                nc.sync.dma_start(stats8[:], cc_out[:])
            else:
                stats8 = small.tile([N_CORES, 2 + H], F32)
                nc.gpsimd.memset(stats8[:], 1.0)


            # ---------------- global combine (replicated) -----------------
            if probe_no_tail:
                ovec0 = small.tile([1, OUT_DIM + 1 + 2 * H], F32)
                nc.gpsimd.memset(ovec0[:], 0.0)
                nc.vector.tensor_copy(ovec0[:, 0:1], stats8[0:1, 0:1])
                nc.sync.dma_start(outv[:], ovec0[:])
            p_t8 = ps.tile([1, N_CORES], F32, tag="ps")
            nc.tensor.transpose(p_t8[:], stats8[:, 0:1], ident[0:N_CORES, 0:N_CORES])
            m_g = small.tile([1, 1], F32)
            nc.vector.tensor_reduce(m_g[:], p_t8[:], axis=AX.X, op=ALU.min)
            p_b8 = ps.tile([N_CORES, 1], F32, tag="ps")
            nc.tensor.matmul(p_b8[:], ones_row[:, 0:N_CORES], m_g[:],
                             start=True, stop=True)
            mg8 = small.tile([N_CORES, 1], F32)
            nc.vector.tensor_copy(mg8[:], p_b8[:])
            scale8 = small.tile([N_CORES, 1], F32)
            nc.scalar.activation(scale8[:], stats8[:, 0:1], AF.Exp,
                                 bias=mg8[:], scale=-1.0)
            s8w = small.tile([N_CORES, 1], F32)
            nc.vector.tensor_mul(s8w[:], stats8[:, 1:2], scale8[:])
            p_S = ps.tile([1, 1], F32, tag="ps")
            nc.tensor.matmul(p_S[:], ones_col[0:N_CORES, :], s8w[:],
                             start=True, stop=True)
            p_Pg = ps.tile([1, H], F32, tag="ps2")
            nc.tensor.matmul(p_Pg[:], scale8[:], stats8[:, 2:2 + H],
                             start=True, stop=True)
            rS = small.tile([1, 1], F32)
            nc.vector.reciprocal(rS[:], p_S[:])
            mt_pre = small.tile([1, H], F32)
            nc.vector.tensor_scalar_mul(mt_pre[:], p_Pg[:], rS[:])
            m_t = small.tile([1, H], F32)
            nc.scalar.activation(m_t[:], mt_pre[:], AF.Tanh)

            # ---------------- finish LSTM + heads -------------------------
            rm = small.tile([1, H], F32)
            nc.vector.tensor_mul(rm[:], r_g, m_t[:])
            c_t = small.tile([1, H], F32)
            nc.vector.tensor_add(c_t[:], c_pre[:], rm[:])
            tct = small.tile([1, H], F32)
            nc.scalar.activation(tct[:], c_t[:], AF.Tanh)
            h_t = small.tile([1, H], F32)
            nc.vector.tensor_mul(h_t[:], o_g, tct[:])

            p_hc2 = ps.tile([128, 1], F32, tag="ps")
            nc.tensor.transpose(p_hc2[:], h_t[:], ident[0:1, 0:1])
            ht_col = small.tile([128, 1], F32)
            nc.vector.tensor_copy(ht_col[:], p_hc2[:])

            p_a = ps.tile([1, OUT_DIM], F32, tag="ps")
            nc.tensor.matmul(p_a[:], ht_col[:], waT[:], start=True, stop=True)
            z_a = small.tile([1, OUT_DIM], F32)
            nc.vector.tensor_add(z_a[:], p_a[:], ba_sb[:])
            mx = small.tile([1, 1], F32)
            nc.vector.tensor_reduce(mx[:], z_a[:], axis=AX.X, op=ALU.max)
            nmx = small.tile([1, 1], F32)
            nc.vector.tensor_scalar_mul(nmx[:], mx[:], -1.0)
            ez = small.tile([1, OUT_DIM], F32)
            nc.scalar.activation(ez[:], z_a[:], AF.Exp, bias=nmx[:])
            sez = small.tile([1, 1], F32)
            nc.vector.tensor_reduce(sez[:], ez[:], axis=AX.X, op=ALU.add)
            rsez = small.tile([1, 1], F32)
            nc.vector.reciprocal(rsez[:], sez[:])
            pi = small.tile([1, OUT_DIM], F32)
            nc.vector.tensor_scalar_mul(pi[:], ez[:], rsez[:])

            p_v = ps.tile([1, 1], F32, tag="ps")
            nc.tensor.matmul(p_v[:], ht_col[:], wcT[:], start=True, stop=True)
            v_t = small.tile([1, 1], F32)
            nc.vector.tensor_add(v_t[:], p_v[:], bc_sb[:])

            ovec = small.tile([1, OUT_DIM + 1 + 2 * H], F32)
            nc.vector.tensor_copy(ovec[:, 0:OUT_DIM], pi[:])
            nc.vector.tensor_copy(ovec[:, OUT_DIM:OUT_DIM + 1], v_t[:])
            nc.vector.tensor_copy(ovec[:, OUT_DIM + 1:OUT_DIM + 1 + H], h_t[:])
            nc.vector.tensor_copy(ovec[:, OUT_DIM + 1 + H:], c_t[:])
            nc.sync.dma_start(outv[:], ovec[:])

    nc.finalize()
    return nc


_NC = None
_RT = None

_NEFF_CACHE_DIR = os.environ.get(
    "DND_NEFF_CACHE", os.path.expanduser("~/.cache/dnd_neff"))


def _get_nc():
    global _NC
    if _NC is None:
        _NC = _build_nc()
    return _NC


def _install_neff_cache():
    """Content-addressed on-disk NEFF cache so fresh processes skip the
    multi-minute walrus compile."""
    from concourse import bass2jax
    if getattr(bass2jax, "_dnd_neff_cache", False):
        return
    orig = bass2jax.compile_bir_kernel

    def cached(bir_json, tmpdir, neff_name="file.neff"):
        try:
            b = bir_json if isinstance(bir_json, bytes) else bir_json.encode()
            key = hashlib.sha256(b).hexdigest()
            os.makedirs(_NEFF_CACHE_DIR, exist_ok=True)
            path = os.path.join(_NEFF_CACHE_DIR, key + ".neff")
            if os.path.exists(path):
                dst = os.path.join(tmpdir, neff_name)
                shutil.copyfile(path, dst)
                return dst
            out = orig(bir_json, tmpdir, neff_name=neff_name)
            tmp = path + ".tmp%d" % os.getpid()
            shutil.copyfile(out, tmp)
            os.replace(tmp, path)
            return out
        except Exception:
            return orig(bir_json, tmpdir, neff_name=neff_name)

    bass2jax.compile_bir_kernel = cached
    bass2jax._dnd_neff_cache = True


class _Runtime:
    pass


def _get_rt():
    """Cached jitted shard_map executable + resident device zero buffers."""
    global _RT
    if _RT is not None:
        return _RT
    import jax
    from jax.sharding import Mesh, PartitionSpec, NamedSharding
    try:
        from jax import shard_map
    except ImportError:
        from jax.experimental.shard_map import shard_map
    from concourse import bass2jax

    _install_neff_cache()
    nc = _get_nc()
    bass2jax.install_neuronx_cc_hook()

    in_names, out_names, out_avals = [], [], []
    for alloc in nc.m.functions[0].allocations:
        if not isinstance(alloc, mybir.MemoryLocationSet):
            continue
        name = alloc.memorylocations[0].name
        if alloc.kind == "ExternalInput":
            in_names.append(name)
        elif alloc.kind == "ExternalOutput":
            out_names.append(name)
            out_avals.append(jax.core.ShapedArray(
                tuple(alloc.tensor_shape), mybir.dt.np(alloc.dtype)))
    part_name = nc.partition_id_tensor.name if nc.partition_id_tensor else None
    if part_name is not None:
        in_names = [n for n in in_names if n != part_name]
    n_params = len(in_names)
    all_names = in_names + out_names + ([part_name] if part_name else [])

    def _body(*args):
        operands = list(args)
        if part_name is not None:
            operands.append(bass2jax.partition_id_tensor())
        return tuple(bass2jax._bass_exec_p.bind(
            *operands,
            out_avals=tuple(out_avals),
            in_names=tuple(all_names),
            out_names=tuple(out_names),
            lowering_input_output_aliases=(),
            sim_require_finite=True,
            sim_require_nnan=True,
            nc=nc,
        ))

    devices = jax.devices()[:N_CORES]
    mesh = Mesh(np.asarray(devices), ("core",))
    spec = PartitionSpec("core")
    sh = NamedSharding(mesh, spec)
    sm_kwargs = dict(mesh=mesh,
                     in_specs=(spec,) * (n_params + len(out_names)),
                     out_specs=(spec,) * len(out_names))
    try:
        smapped = shard_map(_body, check_vma=False, **sm_kwargs)
    except TypeError:
        smapped = shard_map(_body, check_rep=False, **sm_kwargs)
    fn = jax.jit(smapped, keep_unused=True)

    import jax.numpy as jnp
    zeros = []
    for a in out_avals:
        shape = (N_CORES * a.shape[0],) + tuple(a.shape[1:])
        z = jax.jit(lambda s=shape, d=a.dtype: jnp.zeros(s, d),
                    out_shardings=sh)()
        zeros.append(z)
    jax.block_until_ready(zeros)

    rt = _Runtime()
    rt.jax = jax
    rt.fn = fn
    rt.sh = sh
    rt.in_names = in_names
    rt.out_names = out_names
    rt.zeros = zeros
    _RT = rt
    return rt


def _prep_host(x_t, h, c, W_i2h, b_i2h, W_h2h, b_h2h, mem_keys, mem_vals,
               W_actor, b_actor, W_critic, b_critic):
    x = np.ascontiguousarray(np.asarray(x_t, np.float32).reshape(1, D))
    hr = np.ascontiguousarray(np.asarray(h, np.float32).reshape(1, H))
    cr = np.ascontiguousarray(np.asarray(c, np.float32).reshape(1, H))
    keys = np.asarray(mem_keys, np.float32)
    vals = np.asarray(mem_vals, np.float32)

    LP = Q * N_CORES
    keys_p = np.empty((LP, D), np.float32)
    keys_p[:L] = keys
    keys_p[L:] = PAD_VAL
    vals_p = np.empty((LP, H), np.float32)
    vals_p[:L] = vals
    vals_p[L:] = 0.0

    common = dict(
        xr=x, hr=hr, cr=cr,
        wi=np.ascontiguousarray(np.asarray(W_i2h, np.float32)),
        bi=np.ascontiguousarray(np.asarray(b_i2h, np.float32).reshape(1, -1)),
        wh=np.ascontiguousarray(np.asarray(W_h2h, np.float32)),
        bh=np.ascontiguousarray(np.asarray(b_h2h, np.float32).reshape(1, -1)),
        wa=np.ascontiguousarray(np.asarray(W_actor, np.float32)),
        ba=np.ascontiguousarray(np.asarray(b_actor, np.float32).reshape(1, -1)),
        wc=np.ascontiguousarray(np.asarray(W_critic, np.float32).reshape(1, H)),
        bc=np.ascontiguousarray(np.asarray(b_critic, np.float32).reshape(1, 1)),
    )
    return x, keys, vals, keys_p, vals_p, common


def _assemble(ov, x, keys, vals):
    out = np.asarray(ov, np.float32).reshape(-1).copy()
    keys_new = np.empty((L, D), np.float32)
    vals_new = np.empty((L, H), np.float32)
    keys_new[0] = x[0]
    vals_new[0] = out[OUT_DIM + 1 + H:]
    keys_new[1:] = keys[:L - 1]
    vals_new[1:] = vals[:L - 1]
    return out, keys_new, vals_new


def kernel(x_t, h, c, W_i2h, b_i2h, W_h2h, b_h2h, mem_keys, mem_vals,
           W_actor, b_actor, W_critic, b_critic, **_ignored):
    x, keys, vals, keys_p, vals_p, common = _prep_host(
        x_t, h, c, W_i2h, b_i2h, W_h2h, b_h2h, mem_keys, mem_vals,
        W_actor, b_actor, W_critic, b_critic)

    try:
        if os.environ.get("DND_FORCE_FALLBACK"):
            raise RuntimeError("forced fallback")
        rt = _get_rt()
        gi = {"keys": keys_p, "vals": vals_p}
        for k, v in common.items():
            gi[k] = np.concatenate([v] * N_CORES, axis=0)
        args = [rt.jax.device_put(gi[n], rt.sh) for n in rt.in_names]
        outs = rt.fn(*args, *rt.zeros)
        ov = np.asarray(outs[rt.out_names.index("outv")])
        ov = ov.reshape(N_CORES, -1)[0]
        del outs, args
        return _assemble(ov, x, keys, vals)
    except Exception:
        if os.environ.get("DND_DEBUG"):
            import traceback
            traceback.print_exc()

    # fallback: reference-grade path through run_bass_kernel_spmd
    nc = _get_nc()
    in_maps = [
        dict(keys=keys_p[i * Q:(i + 1) * Q], vals=vals_p[i * Q:(i + 1) * Q],
             **common)
        for i in range(N_CORES)
    ]
    res = run_bass_kernel_spmd(nc, in_maps, core_ids=list(range(N_CORES))).results
    ov = np.asarray(res[0]["outv"], np.float32).reshape(-1)
    out = ov.copy()
    keys_new = np.empty((L, D), np.float32)
    vals_new = np.empty((L, H), np.float32)
    keys_new[0] = x[0]
    vals_new[0] = ov[OUT_DIM + 1 + H:]
    pos = 1
    for i in range(N_CORES):
        take = min(Q, L - pos)
        if take <= 0:
            break
        keys_new[pos:pos + take] = res[i]["keys_out"][:take]
        vals_new[pos:pos + take] = res[i]["vals_out"][:take]
        pos += take
    return out, keys_new, vals_new
